# revision 2
# baseline (speedup 1.0000x reference)
"""GAT-style message passing (nn_DynamicGraphStorage) on 8 Trainium2 NeuronCores.

Math reduction vs reference:
  - e = eattr_f @ W_edge is only used via a_edge = e @ att_edge, so
    a_edge = eattr_f @ v_edge with v_edge = W_edge @ att_edge  (per-edge dot).
  - loop_attr @ W_edge @ att_edge = segment_sum(b, dst) / max(deg,1) with
    b_e = edge_attr[e] . v_edge  (segment ops commute with the matvec).
  - softmax max-subtraction cancels exactly in the final ratio, so it is
    dropped (alpha magnitudes are O(few), exp() cannot overflow).

Sharding:
  - nodes are assigned to 8 cores (balanced on in-degree, snake order);
    each core owns its nodes' outputs (dst side).
  - within a core, nodes are bin-packed into NBINS bins of <=128 nodes such
    that every (bin, src-quarter) cell has <= CELL edges.  A quarter is a
    pair of cores (global slot range / 4), so gather indices into the
    AllGathered h table fit in int16.
  - edge stream per core: quarter-major, cell-major, padded to exactly CELL
    edges per cell -> fully uniform SPMD kernel (one module for 8 cores).
  - per 128-edge chunk: one-hot sel (dst one-hot within bin) matmul
    accumulates [ex*h | ex | b] into PSUM; per cell PSUM flushes into an
    SBUF accumulator slot (static address).  h[src] rows come from
    dma_gather (SWDGE) on 4 rotating queues.
"""

import os
import sys

import numpy as np

# ---------------------------------------------------------------- geometry
N_NODES = 100000
N_EDGES = 1600000
HID = 128
NC = 8
NEG_SLOPE = 0.2
EPS = 1e-16

CFG = dict(
    NBINS=104,          # bins per core (128 node slots each)
    CPQ=4,              # chunks per (bin, quarter) cell  -> CELL = 512 edges
    GSZ=8,              # chunks per gather group (1024 idx / dma_gather)
)


def _derived(cfg):
    d = dict(cfg)
    d["SLOTS_PER_CORE"] = d["NBINS"] * 128
    d["TAB_ROWS"] = NC * d["SLOTS_PER_CORE"]
    d["QROWS"] = d["TAB_ROWS"] // 4            # rows per src-quarter
    d["CELL"] = d["CPQ"] * 128                 # edges per (bin, quarter)
    d["CHUNKS_PER_Q"] = d["NBINS"] * d["CPQ"]
    assert d["CHUNKS_PER_Q"] % d["GSZ"] == 0
    d["GROUPS_PER_Q"] = d["CHUNKS_PER_Q"] // d["GSZ"]
    d["GROUPS"] = 4 * d["GROUPS_PER_Q"]
    d["EPC"] = 4 * d["NBINS"] * d["CELL"]      # padded edges per core
    d["NIDX"] = d["GSZ"] * 128                 # idx per gather
    assert d["QROWS"] <= 32768, "gather idx must fit int16"
    return d


# ================================================================ host prep
def _pack_bins(nodes, qdeg, nbins, cell):
    """Greedy-pack `nodes` (with per-quarter in-degree rows qdeg[n]) into
    nbins bins of <=128 nodes with every per-quarter load <= cell.
    Returns bin id and position-in-bin per node."""
    loads = np.zeros((nbins, 4), dtype=np.int64)
    counts = np.zeros(nbins, dtype=np.int64)
    bin_of = np.empty(len(nodes), dtype=np.int64)
    pos_of = np.empty(len(nodes), dtype=np.int64)
    order = np.argsort(-qdeg.sum(1), kind="stable")
    for i in order:
        qd = qdeg[i]
        ok = (counts < 128) & np.all(loads + qd <= cell, axis=1)
        if not ok.any():
            raise RuntimeError("bin packing failed; raise NBINS")
        cand = np.where(ok)[0]
        j = cand[np.argmin(loads[cand].sum(1))]
        bin_of[i] = j
        pos_of[i] = counts[j]
        counts[j] += 1
        loads[j] += qd
    return bin_of, pos_of


def _prep(edge_attr, emb_table, edge_index, cfg):
    d = _derived(cfg)
    NB, CELL, CPQ = d["NBINS"], d["CELL"], d["CPQ"]
    SPC = d["SLOTS_PER_CORE"]

    src = np.asarray(edge_index[0], dtype=np.int64)
    dst = np.asarray(edge_index[1], dtype=np.int64)
    deg = np.bincount(dst, minlength=N_NODES)

    # --- assign nodes to cores: snake over degree-sorted nodes
    nd_order = np.argsort(-deg, kind="stable")
    snake = np.tile(np.r_[np.arange(NC), np.arange(NC)[::-1]],
                    (N_NODES + 2 * NC - 1) // (2 * NC))[:N_NODES]
    core_of = np.empty(N_NODES, dtype=np.int64)
    core_of[nd_order] = snake
    quarter_of_node = core_of // 2             # src-quarter of a node

    # --- per-core bin packing on 4-vector of per-quarter in-degree
    src_q = quarter_of_node[src]
    qdeg = np.zeros((N_NODES, 4), dtype=np.int64)
    np.add.at(qdeg, (dst, src_q), 1)

    slot_in_core = np.empty(N_NODES, dtype=np.int64)
    nodes_per_core = []
    for c in range(NC):
        nodes_c = np.where(core_of == c)[0]
        bin_of, pos_of = _pack_bins(nodes_c, qdeg[nodes_c], NB, CELL)
        slot_in_core[nodes_c] = bin_of * 128 + pos_of
        nodes_per_core.append(nodes_c)

    gslot = core_of * SPC + slot_in_core       # global permuted row

    # --- per-core padded edge streams
    ecore = core_of[dst]
    ebin = slot_in_core[dst] // 128
    equarter = src_q
    cell_id = (ecore * 4 + equarter) * NB + ebin     # global cell id
    n_cells = NC * 4 * NB

    order = np.argsort(cell_id, kind="stable")
    cell_sorted = cell_id[order]
    counts = np.bincount(cell_id, minlength=n_cells)
    assert counts.max() <= CELL, (counts.max(), CELL)
    starts = np.zeros(n_cells, dtype=np.int64)
    starts[1:] = np.cumsum(counts)[:-1]
    rank = np.arange(len(order)) - starts[cell_sorted]
    pos = cell_sorted * CELL + rank            # position in global padded stream
    eid_stream = np.full(n_cells * CELL, -1, dtype=np.int64)
    eid_stream[pos] = order

    # per-core views [4*NB*CELL]
    per_core = eid_stream.reshape(NC, 4 * NB * CELL)

    host = {"d": d, "core_of": core_of, "slot_in_core": slot_in_core,
            "nodes_per_core": nodes_per_core, "deg": deg}

    # --- device input arrays per core
    ins = []
    G, GSZ, NIDX = d["GROUPS"], d["GSZ"], d["NIDX"]
    for c in range(NC):
        eids = per_core[c]
        pad = eids < 0
        e_safe = np.where(pad, 0, eids)

        ea = edge_attr[e_safe].astype(np.float32, copy=True)
        ea[pad] = 0.0
        # wrap: [G, 128 lanes, GSZ chunks, HID]
        ea = ea.reshape(G, GSZ, 128, HID).transpose(0, 2, 1, 3).copy()

        dw = (slot_in_core[dst[e_safe]] % 128).astype(np.int16)
        dw[pad] = -1
        dw = dw.reshape(G, GSZ, 128).transpose(0, 2, 1).copy()

        q_of_group = np.repeat(np.arange(4), d["GROUPS_PER_Q"])
        gi = (gslot[src[e_safe]] - np.repeat(q_of_group, NIDX) * d["QROWS"])
        gi = np.where(pad, 0, gi)
        assert gi.min() >= 0 and gi.max() < 32768
        gi = gi.astype(np.int16).reshape(G, NIDX)
        # idx layout: [16, NIDX/16] F-order, tiled to 128 partitions
        gi = gi.reshape(G, NIDX // 16, 16).transpose(0, 2, 1)
        gi = np.tile(gi, (1, 8, 1)).copy()

        nodes_c = nodes_per_core[c]
        sl = slot_in_core[nodes_c]
        x_loc = np.zeros((SPC, HID), dtype=np.float32)
        x_loc[sl] = emb_table[nodes_c]
        invdeg = np.ones(SPC, dtype=np.float32)
        invdeg[sl] = 1.0 / np.maximum(deg[nodes_c], 1.0)

        ins.append({
            "eattr_w": ea,                       # [G,128,GSZ,HID] f32
            "dstw": dw,                          # [G,128,GSZ] i16
            "gidx": gi,                          # [G,128,NIDX/16] i16
            "x_locT": np.ascontiguousarray(x_loc.T),   # [HID, SPC] f32
            "invdeg2d": np.ascontiguousarray(
                invdeg.reshape(NB, 128).T),      # [128, NB] f32
        })
    return host, ins


# ================================================================ device
def _build_module(cfg):
    sys.path.insert(0, "/opt/trn_rl_repo")
    import concourse.bacc as bacc
    import concourse.mybir as mybir
    import concourse.tile as tile
    from concourse.library_config import mlp
    from concourse.masks import make_identity

    d = _derived(cfg)
    NB, CPQ, GSZ = d["NBINS"], d["CPQ"], d["GSZ"]
    SPC, TAB, QR = d["SLOTS_PER_CORE"], d["TAB_ROWS"], d["QROWS"]
    G, GPQ, NIDX = d["GROUPS"], d["GROUPS_PER_Q"], d["NIDX"]
    FP = mybir.dt.float32
    I16 = mybir.dt.int16
    AF = mybir.ActivationFunctionType
    OP = mybir.AluOpType

    stage = int(os.environ.get("KDEBUG_STAGE", "4"))
    esub = int(os.environ.get("KDEBUG_ESUB", "5"))
    nc = bacc.Bacc("TRN2", num_swdge_queues=4)

    # ---- I/O
    eattr_w = nc.dram_tensor("eattr_w", [G, 128, GSZ, HID], FP, kind="ExternalInput")
    dstw_d = nc.dram_tensor("dstw", [G, 128, GSZ], I16, kind="ExternalInput")
    gidx_d = nc.dram_tensor("gidx", [G, 128, NIDX // 16], I16, kind="ExternalInput")
    x_locT = nc.dram_tensor("x_locT", [HID, SPC], FP, kind="ExternalInput")
    invdeg_d = nc.dram_tensor("invdeg2d", [128, NB], FP, kind="ExternalInput")
    W_d = nc.dram_tensor("W", [HID, HID], FP, kind="ExternalInput")
    WeT_d = nc.dram_tensor("WeT", [HID, HID], FP, kind="ExternalInput")
    asrc_c = nc.dram_tensor("att_src", [HID, 1], FP, kind="ExternalInput")
    adst_c = nc.dram_tensor("att_dst", [HID, 1], FP, kind="ExternalInput")
    aedge_c = nc.dram_tensor("att_edge", [HID, 1], FP, kind="ExternalInput")
    bias_c = nc.dram_tensor("bias", [HID, 1], FP, kind="ExternalInput")
    out_d = nc.dram_tensor("out_bins", [SPC, HID], FP, kind="ExternalOutput")

    h_loc = nc.dram_tensor("h_loc", [SPC, HID], FP)
    h_tab = nc.dram_tensor("h_tab", [TAB, HID], FP, addr_space="Shared")

    with tile.TileContext(nc) as tc:
        with (
            tc.tile_pool(name="const", bufs=1) as cp,
            tc.tile_pool(name="node", bufs=3) as np_,
            tc.tile_pool(name="npsum", bufs=1, space="PSUM") as npp,
            tc.tile_pool(name="edge", bufs=3) as ep,
            tc.tile_pool(name="cell", bufs=2) as cellp,
            tc.tile_pool(name="epsum", bufs=2, space="PSUM") as epp,
            tc.tile_pool(name="fin", bufs=3) as fp_,
        ):
            ident = cp.tile([128, 128], FP)
            make_identity(nc, ident[:])
            W_sb = cp.tile([HID, HID], FP)
            nc.sync.dma_start(W_sb[:], W_d[:])
            WeT_sb = cp.tile([HID, HID], FP)
            nc.sync.dma_start(WeT_sb[:], WeT_d[:])
            asrc_col = cp.tile([HID, 1], FP)
            nc.sync.dma_start(asrc_col[:], asrc_c[:])
            adst_col = cp.tile([HID, 1], FP)
            nc.sync.dma_start(adst_col[:], adst_c[:])
            aedge_col = cp.tile([HID, 1], FP)
            nc.sync.dma_start(aedge_col[:], aedge_c[:])
            bias_col = cp.tile([HID, 1], FP)
            nc.sync.dma_start(bias_col[:], bias_c[:])
            invdeg_sb = cp.tile([128, NB], FP)
            nc.sync.dma_start(invdeg_sb[:], invdeg_d[:])
            iota_sb = cp.tile([128, 128], I16)
            nc.gpsimd.iota(iota_sb[:], pattern=[[1, 128]], base=0,
                           channel_multiplier=0)
            nc.gpsimd.load_library(mlp)

            # v_edge = W_edge @ att_edge  (WeT.T @ att_edge), replicated rows
            vz = npp.tile([HID, 1], FP, tag="misc")
            nc.tensor.matmul(vz[:], lhsT=WeT_sb[:], rhs=aedge_col[:],
                             start=True, stop=True)
            v_col = cp.tile([HID, 1], FP)
            nc.scalar.copy(v_col[:], vz[:])
            vrep_ps = npp.tile([128, 128], FP, tag="misc")
            nc.tensor.transpose(vrep_ps[:], v_col[:].to_broadcast([128, 128]),
                                ident[:])
            v_rep = cp.tile([128, 128], FP)
            nc.scalar.copy(v_rep[:], vrep_ps[:])
            # att_src replicated rows (R slot 0); R slot 1 = a_dst bin rep
            R = cp.tile([128, 2, 256], FP)  # padded so the 3D AP stays unmerged
            arep_ps = npp.tile([128, 128], FP, tag="misc")
            nc.tensor.transpose(arep_ps[:], asrc_col[:].to_broadcast([128, 128]),
                                ident[:])
            nc.scalar.copy(R[:, 0, 0:128], arep_ps[:])
            brep_ps = npp.tile([128, 128], FP, tag="misc")
            nc.tensor.transpose(brep_ps[:], bias_col[:].to_broadcast([128, 128]),
                                ident[:])
            bias_rep = cp.tile([128, 128], FP)
            nc.scalar.copy(bias_rep[:], brep_ps[:])

            asrc_sb = cp.tile([128, NB], FP)
            adst_sb = cp.tile([128, NB], FP)
            accum = cp.tile([128, NB * 130], FP)
            nc.vector.memset(accum[:], 0.0)

            # ---------------- node phase: h, a_src, a_dst per 128-node tile
            for t in range(NB):
                xT = np_.tile([HID, 128], FP, tag="xT")
                nc.sync.dma_start(xT[:], x_locT[:, t * 128:(t + 1) * 128])
                hT_ps = npp.tile([HID, 128], FP, tag="hT")
                nc.tensor.matmul(hT_ps[:], lhsT=W_sb[:], rhs=xT[:],
                                 start=True, stop=True)
                hT = np_.tile([HID, 128], FP, tag="hTs")
                nc.scalar.copy(hT[:], hT_ps[:])
                a1 = npp.tile([128, 1], FP, tag="a")
                nc.tensor.matmul(a1[:], lhsT=hT[:], rhs=asrc_col[:],
                                 start=True, stop=True)
                nc.scalar.copy(asrc_sb[:, t:t + 1], a1[:])
                a2 = npp.tile([128, 1], FP, tag="a")
                nc.tensor.matmul(a2[:], lhsT=hT[:], rhs=adst_col[:],
                                 start=True, stop=True)
                nc.scalar.copy(adst_sb[:, t:t + 1], a2[:])
                h_ps = npp.tile([128, HID], FP, tag="hN")
                nc.tensor.transpose(h_ps[:], hT[:], ident[:])
                h_sb = np_.tile([128, HID], FP, tag="hNs")
                nc.scalar.copy(h_sb[:], h_ps[:])
                nc.sync.dma_start(h_loc[t * 128:(t + 1) * 128, :], h_sb[:])

            sumad = cp.tile([128, NB], FP)
            nc.vector.tensor_add(sumad[:], asrc_sb[:], adst_sb[:])

            # ---------------- AllGather h -> shared table
            if stage >= 2:
              nc.gpsimd.collective_compute(
                "AllGather", OP.bypass,
                ins=[h_loc.ap()],
                outs=[h_tab.ap()],
                replica_groups=[list(range(NC))],
              )

            # ---------------- edge phase
            for g in range(G if stage >= 3 else 0):
                q = g // GPQ
                gi = ep.tile([128, NIDX // 16], I16, tag="gi")
                nc.sync.dma_start(gi[:], gidx_d[g])
                dw = ep.tile([128, GSZ], I16, tag="dw")
                nc.sync.dma_start(dw[:], dstw_d[g])
                ES = ep.tile([128, GSZ, HID], FP, tag="ES")
                nc.sync.dma_start(ES[:], eattr_w[g])
                GS = ep.tile([128, 2 * GSZ, HID], FP, tag="GS")
                if stage >= 4:
                    nc.gpsimd.dma_gather(
                        GS[:, 0:GSZ, :], h_tab[q * QR:(q + 1) * QR, :], gi[:],
                        NIDX, NIDX, HID, queue_num=g % 4)
                else:
                    nc.vector.memset(GS[:, 0:GSZ, :], 0.0)
                rhs = ep.tile([128, GSZ, 130], FP, tag="rhs")
                cols = ep.tile([128, GSZ, 3], FP, tag="cols")  # alpha, abs, t2
                junk = ep.tile([128, 2, 128], FP, tag="junk")
                junk1 = ep.tile([128, 128], FP, tag="junk1")
                GSr = GS[:].rearrange("p (a e) f -> p e a f", a=2)

                for s in range(GSZ):
                    chunk_q = (g % GPQ) * GSZ + s
                    cell = chunk_q // CPQ
                    cpos = chunk_q % CPQ
                    if cpos == 0:
                        ad_ps = epp.tile([128, 128], FP, tag="adrep")
                        nc.tensor.transpose(
                            ad_ps[:],
                            adst_sb[:, cell:cell + 1].to_broadcast([128, 128]),
                            ident[:])
                        nc.scalar.copy(R[:, 1, 0:128], ad_ps[:])
                        ps = epp.tile([128, 130], FP, tag="cellps")
                    # b = eattr . v  -> rhs col 129
                    if esub >= 1:
                        nc.vector.scalar_tensor_tensor(
                            out=junk1[:], in0=ES[:, s, :], scalar=1.0,
                            in1=v_rep[:], op0=OP.mult, op1=OP.mult,
                            accum_out=rhs[:, s, 129:130])
                    else:
                        nc.vector.memset(rhs[:, s, 129:130], 0.0)
                    # sel one-hot into GS slot GSZ+s
                    if esub >= 2:
                        nc.vector.tensor_tensor(
                            out=GS[:, GSZ + s, :],
                            in0=dw[:, s:s + 1].to_broadcast([128, 128]),
                            in1=iota_sb[:], op=OP.is_equal)
                    else:
                        nc.vector.memset(GS[:, GSZ + s, :], 0.0)
                    # t2 = h.att_src + sel.adst_bin; alpha = b + t2
                    if esub >= 3:
                        nc.vector.scalar_tensor_tensor(
                            out=junk[:], in0=GSr[:, s], scalar=1.0,
                            in1=R[:, 0:2, 0:128], op0=OP.mult, op1=OP.mult,
                            accum_out=cols[:, s, 2:3])
                        nc.scalar.activation(
                            cols[:, s, 0:1], cols[:, s, 2:3], AF.Identity,
                            bias=rhs[:, s, 129:130])
                    else:
                        nc.vector.memset(cols[:, s, 0:1], 0.0)
                    # lrelu(a) = 0.6a + 0.4|a|; ex = exp(lrelu) -> rhs col 128
                    if esub >= 4:
                        nc.scalar.activation(
                            cols[:, s, 1:2], cols[:, s, 0:1], AF.Abs,
                            scale=(1.0 - NEG_SLOPE) / 2.0)
                        nc.scalar.activation(
                            rhs[:, s, 128:129], cols[:, s, 0:1], AF.Exp,
                            scale=(1.0 + NEG_SLOPE) / 2.0, bias=cols[:, s, 1:2])
                        # msg = ex * h -> rhs cols 0:128
                        nc.scalar.activation(
                            rhs[:, s, 0:128], GS[:, s, :], AF.Copy,
                            scale=rhs[:, s, 128:129])
                    else:
                        nc.vector.memset(rhs[:, s, 128:129], 0.0)
                        nc.vector.memset(rhs[:, s, 0:128], 0.0)
                    nc.tensor.matmul(ps[:], lhsT=GS[:, GSZ + s, :],
                                     rhs=rhs[:, s, :],
                                     start=(cpos == 0), stop=(cpos == CPQ - 1))
                    if cpos == CPQ - 1:
                        b = cell
                        nc.vector.tensor_add(
                            accum[:, b * 130:(b + 1) * 130],
                            accum[:, b * 130:(b + 1) * 130], ps[:])

            # ---------------- final: self loops, divide, bias
            for b in range(NB):
                acc = accum[:, b * 130:(b + 1) * 130]
                hb = fp_.tile([128, HID], FP, tag="hb")
                nc.sync.dma_start(hb[:], h_loc[b * 128:(b + 1) * 128, :])
                t0 = fp_.tile([128, 4], FP, tag="t0")
                # t0[:,0] = bsum * invdeg ; alpha_loop = lrelu(t0 + sumad)
                nc.vector.tensor_tensor(
                    t0[:, 0:1], acc[:, 129:130], invdeg_sb[:, b:b + 1],
                    op=OP.mult)
                nc.vector.tensor_add(t0[:, 0:1], t0[:, 0:1],
                                     sumad[:, b:b + 1])
                nc.scalar.activation(t0[:, 1:2], t0[:, 0:1], AF.Abs,
                                     scale=(1.0 - NEG_SLOPE) / 2.0)
                nc.scalar.activation(t0[:, 2:3], t0[:, 0:1], AF.Exp,
                                     scale=(1.0 + NEG_SLOPE) / 2.0,
                                     bias=t0[:, 1:2])
                # denom = dsum + ex_loop + EPS ; rec = 1/denom
                nc.vector.tensor_add(t0[:, 3:4], acc[:, 128:129], t0[:, 2:3])
                nc.vector.tensor_scalar_add(t0[:, 3:4], t0[:, 3:4], EPS)
                rec = fp_.tile([128, 1], FP, tag="rec")
                nc.vector.reciprocal(rec[:], t0[:, 3:4])
                # msum += ex_loop * h ; out = msum * rec + bias
                m1 = fp_.tile([128, HID], FP, tag="m1")
                nc.scalar.activation(m1[:], hb[:], AF.Copy, scale=t0[:, 2:3])
                nc.vector.tensor_add(m1[:], m1[:], acc[:, 0:128])
                nc.scalar.activation(m1[:], m1[:], AF.Copy, scale=rec[:])
                ob = fp_.tile([128, HID], FP, tag="ob")
                nc.vector.tensor_add(ob[:], m1[:], bias_rep[:])
                nc.sync.dma_start(out_d[b * 128:(b + 1) * 128, :], ob[:])

    nc.compile()
    return nc


_MODULE_CACHE = {}


def kernel(edge_attr, emb_table, W, W_edge, att_src, att_dst, att_edge, bias,
           edge_index, entity_count):
    cfg = CFG
    d = _derived(cfg)
    edge_attr = np.asarray(edge_attr, dtype=np.float32)
    emb_table = np.asarray(emb_table, dtype=np.float32)

    host, ins = _prep(edge_attr, emb_table, edge_index, cfg)

    key = tuple(sorted(cfg.items()))
    if key not in _MODULE_CACHE:
        _MODULE_CACHE[key] = _build_module(cfg)
    nc = _MODULE_CACHE[key]

    common = {
        "W": np.asarray(W, np.float32),
        "WeT": np.ascontiguousarray(np.asarray(W_edge, np.float32).T),
        "att_src": np.asarray(att_src, np.float32).reshape(HID, 1),
        "att_dst": np.asarray(att_dst, np.float32).reshape(HID, 1),
        "att_edge": np.asarray(att_edge, np.float32).reshape(HID, 1),
        "bias": np.asarray(bias, np.float32).reshape(HID, 1),
    }
    in_maps = [dict(ins[c], **common) for c in range(NC)]

    sys.path.insert(0, "/opt/trn_rl_repo")
    from concourse import bass_utils
    trace = os.environ.get("KERNEL_TRACE", "0") == "1"
    if trace:
        sys.path.insert(0, "/root/problem/work")
        try:
            import tracehook
            tracehook.install()
        except Exception:
            trace = False
    res = bass_utils.run_bass_kernel_spmd(
        nc, in_maps, core_ids=list(range(NC)), trace=trace,
        tmpdir=os.environ.get("KERNEL_TRACE_DIR") or None)
    kernel.last_exec_ns = res.exec_time_ns
    kernel.last_result = res

    out = np.empty((N_NODES, HID), dtype=np.float32)
    for c in range(NC):
        nodes_c = host["nodes_per_core"][c]
        sl = host["slot_in_core"][nodes_c]
        out[nodes_c] = res.results[c]["out_bins"][sl]
    return out


kernel.last_exec_ns = None

